# revision 1
# baseline (speedup 1.0000x reference)
"""Trainium2 Bass kernel for nn_KANLayer (piecewise-constant KAN forward).

Math: reference computes out[b,t,i] = sum_j sum_k basis[b,t,j,k] * coeffs[i,j,k]
where basis k is 1 iff t[max(0,k-2)] <= x < t[min(k+1,9)] with t = linspace(0,1,10).
For x in segment s (t[s] <= x < t[s+1], s in 0..8) exactly k in {s, s+1, s+2} fire, so
    out[t,i] = sum_j D[i,j,seg(x_tj)],  D[i,j,s] = c_s + c_{s+1} + c_{s+2}.
Telescoping over s (step_s = [seg >= s], s=1..8):
    out[t,i] = base[i] + sum_{s=1..8} step_s[t,:] @ G_s[:,i]
with G_s = c_{s+2} - c_{s-1} and base[i] = sum_j D[i,j,0].
This is a K=512*8=4096 matmul with an exact 0/1 bf16 left operand — the minimal
contraction size for this op (9-valued selection -> rank 8 + constant).

Sharding: data-parallel over the 8*2048=16384 tokens, 2048 per NeuronCore;
G/base replicated. Per core: DVE builds 0/1 step planes from host-computed
(bit-exact vs reference) segment indices; PE accumulates 32 K-chunk matmuls
[128x128]x[128x512] bf16 per 128-token tile into PSUM; DVE adds base during
PSUM evacuation.

Perf notes (from ntff traces): MMs pipeline at 216ns (bf16 roofline); the
K-chunk order c = jc*8 + (s-1) + chunk-major accumulation over 4 PSUM banks
keeps early PE demand to the first seg/g slices while the (partition-major,
large-descriptor) DMAs stream in on both HWDGE rings; warmup matmuls on a
scratch tile un-throttle the PE clock (HAM) before real work arrives.
"""

from contextlib import ExitStack

import numpy as np
import ml_dtypes

import concourse.bass as bass  # noqa: F401
import concourse.tile as tile
from concourse import bacc, mybir
from concourse.bass_utils import run_bass_kernel_spmd

N_CORES = 8
TOK_PER_CORE = 2048
IN_F = 512
OUT_F = 512
N_STEP = 8          # step planes s=1..8
JC = IN_F // 128    # 4 j-chunks of 128
KC = N_STEP * JC    # 32 K-chunks of 128
N_GROUPS = 4        # token groups per core
GTOK = TOK_PER_CORE // N_GROUPS   # 512 tokens per group
TT_PER_G = GTOK // 128            # 4 token tiles per group
BF16 = mybir.dt.bfloat16
F32 = mybir.dt.float32

_PROGRAM_CACHE = {}


def _build_program():
    nc = bacc.Bacc("TRN2", target_bir_lowering=False, debug=False)

    # Partition-major layouts: one DMA moves a long contiguous per-partition run.
    seg_d = nc.dram_tensor("seg", [128, JC, TOK_PER_CORE], BF16, kind="ExternalInput").ap()
    g_d = nc.dram_tensor("g", [128, KC, OUT_F], BF16, kind="ExternalInput").ap()
    base_d = nc.dram_tensor("base", [128, OUT_F], F32, kind="ExternalInput").ap()
    out_d = nc.dram_tensor(
        "out", [TOK_PER_CORE // 128, 128, OUT_F], F32, kind="ExternalOutput"
    ).ap()

    with tile.TileContext(nc) as tc, ExitStack() as ctx:
        seg_pool = ctx.enter_context(tc.tile_pool(name="seg", bufs=1))
        g_pool = ctx.enter_context(tc.tile_pool(name="g", bufs=1))
        base_pool = ctx.enter_context(tc.tile_pool(name="base", bufs=1))
        wm_pool = ctx.enter_context(tc.tile_pool(name="wm", bufs=1))
        step_pool = ctx.enter_context(tc.tile_pool(name="step", bufs=2))
        out_pool = ctx.enter_context(tc.tile_pool(name="out", bufs=4))
        # PSUM budget is 8 banks: group-0 accumulates chunk-major across 4
        # banks (bufs=1), later groups go token-tile-major on 1-bank tiles
        # (bufs=4 so evacuation overlaps the next tile's accumulation).
        psum_pool = ctx.enter_context(tc.tile_pool(name="psum", bufs=1, space="PSUM"))
        psum1_pool = ctx.enter_context(tc.tile_pool(name="psum1", bufs=4, space="PSUM"))

        # --- PE warmup: matmuls on a zeroed scratch tile, no DMA dependency.
        # Keeps the PE HAM activity window busy from t=0 so the clock is at
        # 2.4 GHz when the real matmuls start (~3.4us warm-up budget).
        wm = wm_pool.tile([128, 384], BF16, name="wm")
        nc.vector.memset(wm[:], 0.0)
        ps_w = psum_pool.tile([128, TT_PER_G, 512], F32, name="ps")
        for _ in range(20):
            nc.tensor.matmul(
                ps_w[:, 0, :256], wm[:, :128], wm[:, 128:384],
                start=True, stop=True, skip_group_check=True,
            )

        # --- inputs: seg pieces on the sync HWDGE ring, g groups on the
        # scalar HWDGE ring (two independent FIFOs -> parallel streams).
        # seg is split per (group, j-chunk) so the first 128KB piece lands
        # ~1us in and the first step planes can build immediately.
        seg_ts = [
            seg_pool.tile([128, TOK_PER_CORE], BF16, name=f"seg{jc}")
            for jc in range(JC)
        ]
        for q in range(N_GROUPS):
            for jc in range(JC):
                sl = slice(q * GTOK, (q + 1) * GTOK)
                nc.sync.dma_start(seg_ts[jc][:, sl], seg_d[:, jc, sl])

        g_t = g_pool.tile([128, KC, OUT_F], BF16, name="g")
        GSTEP = 4  # chunks per DMA: [128, 4*512] bf16 = 4KB/partition runs
        for gg in range(KC // GSTEP):
            nc.scalar.dma_start(
                g_t[:, gg * GSTEP : (gg + 1) * GSTEP, :],
                g_d[:, gg * GSTEP : (gg + 1) * GSTEP, :],
            )

        base_t = base_pool.tile([128, OUT_F], F32, name="base")
        nc.gpsimd.dma_start(base_t[:], base_d[:])

        # --- main loop: for each 512-token group, build the 32 step planes
        # (chunk c = jc*8 + (s-1): j-chunk-major so early chunks only need
        # seg[0]), then accumulate chunk-major across 4 PSUM banks.
        for q in range(N_GROUPS):
            step_t = step_pool.tile([128, KC, GTOK], BF16, name="step")
            for c in range(KC):
                jc, s = divmod(c, N_STEP)
                nc.vector.tensor_scalar(
                    step_t[:, c, :],
                    seg_ts[jc][:, q * GTOK : (q + 1) * GTOK],
                    float(s + 1) - 0.5,
                    None,
                    mybir.AluOpType.is_ge,
                )
            if q == 0:
                # chunk-major: early matmuls only need the first g/seg slices,
                # so the PE can start while the input DMAs are still streaming.
                ps = psum_pool.tile([128, TT_PER_G, 512], F32, name="ps")
                for c in range(KC):
                    for tt in range(TT_PER_G):
                        nc.tensor.matmul(
                            ps[:, tt, :],
                            step_t[:, c, tt * 128 : (tt + 1) * 128],
                            g_t[:, c, :],
                            start=(c == 0),
                            stop=(c == KC - 1),
                        )
                for tt in range(TT_PER_G):
                    ot = out_pool.tile([128, OUT_F], F32, name="ot")
                    nc.vector.tensor_add(ot[:], ps[:, tt, :], base_t[:])
                    eng = nc.sync if tt % 2 == 0 else nc.scalar
                    eng.dma_start(out_d[q * TT_PER_G + tt], ot[:])
            else:
                # token-tile-major: each tile's accumulation finishes early so
                # its evacuation + output DMA overlap the next tile's matmuls
                # (keeps the post-last-matmul tail to a single tile).
                for tt in range(TT_PER_G):
                    ps1 = psum1_pool.tile([128, 512], F32, name="ps1")
                    for c in range(KC):
                        nc.tensor.matmul(
                            ps1[:],
                            step_t[:, c, tt * 128 : (tt + 1) * 128],
                            g_t[:, c, :],
                            start=(c == 0),
                            stop=(c == KC - 1),
                        )
                    ot = out_pool.tile([128, OUT_F], F32, name="ot")
                    nc.vector.tensor_add(ot[:], ps1[:], base_t[:])
                    eng = nc.sync if tt % 2 == 0 else nc.scalar
                    eng.dma_start(out_d[q * TT_PER_G + tt], ot[:])

    nc.compile()
    return nc


def _get_program():
    if "nc" not in _PROGRAM_CACHE:
        _PROGRAM_CACHE["nc"] = _build_program()
    return _PROGRAM_CACHE["nc"]


def kernel(x: np.ndarray, coeffs: np.ndarray) -> np.ndarray:
    assert x.shape == (8, 2048, IN_F) and coeffs.shape == (OUT_F, IN_F, 12)
    t = np.linspace(0.0, 1.0, 10, dtype=np.float32)  # exact same knots as reference

    # Segment index per element via the same float32 comparisons the
    # reference uses (bit-exact segment assignment).
    xf = np.ascontiguousarray(x.reshape(-1, IN_F))  # [16384, 512]
    seg = np.zeros(xf.shape, dtype=np.float32)
    for m in range(1, 9):
        seg += (xf >= t[m]).astype(np.float32)
    segT = seg.T  # [512 j, 16384 tok]

    c = coeffs.astype(np.float32)
    # G[s-1][j, i] = c[i,j,s+2] - c[i,j,s-1]
    G = np.empty((N_STEP, IN_F, OUT_F), dtype=np.float32)
    for s in range(1, N_STEP + 1):
        G[s - 1] = (c[:, :, s + 2] - c[:, :, s - 1]).T
    # device layout g[p, c, i] with chunk c = jc*8 + (s-1), row p = j - jc*128
    g_dev = np.ascontiguousarray(
        G.reshape(N_STEP, JC, 128, OUT_F).transpose(2, 1, 0, 3).reshape(128, KC, OUT_F)
    ).astype(ml_dtypes.bfloat16)

    base = (c[:, :, 0] + c[:, :, 1] + c[:, :, 2]).sum(axis=1).astype(np.float32)
    base_tile = np.ascontiguousarray(np.broadcast_to(base, (128, OUT_F)))

    # device layout seg[p, jc, t] with row p = j - jc*128
    segT_dev = np.ascontiguousarray(
        segT.reshape(JC, 128, N_CORES * TOK_PER_CORE).transpose(1, 0, 2)
    ).astype(ml_dtypes.bfloat16)

    in_maps = []
    for core in range(N_CORES):
        sl = slice(core * TOK_PER_CORE, (core + 1) * TOK_PER_CORE)
        in_maps.append(
            {
                "seg": np.ascontiguousarray(segT_dev[:, :, sl]),
                "g": g_dev,
                "base": base_tile,
            }
        )

    nc = _get_program()
    res = run_bass_kernel_spmd(nc, in_maps, core_ids=list(range(N_CORES)))
    out = np.stack(
        [res.results[core]["out"].reshape(TOK_PER_CORE, OUT_F) for core in range(N_CORES)]
    )
    return out.astype(np.float32)



# revision 4
# speedup vs baseline: 1.2045x; 1.2045x over previous
"""Trainium2 Bass kernel for nn_KANLayer (piecewise-constant KAN forward).

Math: out[b,t,i] = sum_j D[i,j,seg(x_tj)] with D[i,j,s] = c_s+c_{s+1}+c_{s+2},
seg = which of the 9 knot intervals x falls in. Telescoping over s with step
planes step_s[t,j] = [seg >= s] gives a K=512*8=4096 matmul with a 0/1 left
operand plus a free per-i constant (base).

This version is a hybrid-precision rewrite of the bf16 baseline (127.9us,
PE-bound at 109us of bf16 matmul):
  * j-lanes 0..255 run in fp8e4m3 with DoubleRow perf mode (2 K-lanes per
    PE cell per cycle -> half the matmuls), with error-feedback quantization
    of the cumulative G_s = D_s - D_{s-1} increments and a mean-centering
    correction folded into base. j-lanes 256..511 stay exact bf16.
    Measured (exact inputs, fixed seed): rel err 1.35e-2 < 2e-2 gate.
  * g-stationary orientation: out[i-chunk, token] so each LDWEIGHTS is
    amortized over 2 matmuls and the moving operand is the 512-token step
    plane (N=512 streams).
  * PE work: 128 DoubleRow MMs + 256 bf16 MMs per core ~= 86us vs 109us.

Sharding: data-parallel, 2048 tokens per core; g/base replicated. Output is
produced transposed ([i, token]) and untransposed on the host.
"""

from contextlib import ExitStack

import numpy as np
import ml_dtypes

import concourse.bass as bass  # noqa: F401
import concourse.tile as tile
from concourse import bacc, mybir
from concourse.bass_utils import run_bass_kernel_spmd

N_CORES = 8
TOK = 2048          # tokens per core
IN_F = 512
OUT_F = 512
GTOK = 512          # tokens per group
N_GRP = TOK // GTOK  # 4
IC = 4              # i-feature chunks of 128
JB8 = 2             # j-blocks (of 128) handled in fp8 (f = JB8/4 of lanes)
NC8 = JB8 * 4       # fp8 DoubleRow chunks: 4 s-pairs per fp8 j-block = 8
NCB = (4 - JB8) * 8  # bf16 chunks: 8 s-levels per bf16 j-block = 16
BF16 = mybir.dt.bfloat16
F8 = mybir.dt.float8e4
F32 = mybir.dt.float32
E4NP = ml_dtypes.float8_e4m3

_PROGRAM_CACHE = {}


def _build_program():
    nc = bacc.Bacc("TRN2", target_bir_lowering=False, debug=False)

    seg_d = nc.dram_tensor("seg", [128, 4, TOK], F8, kind="ExternalInput").ap()
    g8_d = nc.dram_tensor("g8", [128, NC8, 2, IC, 128], F8, kind="ExternalInput").ap()
    gbf_d = nc.dram_tensor("gbf", [128, NCB, IC, 128], BF16, kind="ExternalInput").ap()
    base_d = nc.dram_tensor("base", [128, IC], F32, kind="ExternalInput").ap()
    # out[ic, p, tok] -> feature i = ic*128 + p
    out_d = nc.dram_tensor("out", [IC, 128, TOK], F32, kind="ExternalOutput").ap()

    # chunk schedule: bf16 chunks first (cheap DVE builds sprint ahead of the
    # PE), fp8 DoubleRow chunks after. 24 chunks total accumulate per bank.
    chunk_order = [("bf", c) for c in range(NCB)] + [("f8", c) for c in range(NC8)]

    with tile.TileContext(nc) as tc, ExitStack() as ctx:
        seg_pool = ctx.enter_context(tc.tile_pool(name="seg", bufs=1))
        g8_pool = ctx.enter_context(tc.tile_pool(name="g8", bufs=1))
        gbf_pool = ctx.enter_context(tc.tile_pool(name="gbf", bufs=1))
        base_pool = ctx.enter_context(tc.tile_pool(name="base", bufs=1))
        wm_pool = ctx.enter_context(tc.tile_pool(name="wm", bufs=1))
        # all 4 groups' step planes stay resident (distinct tags, bufs=1) ->
        # no WAR stalls at the super boundary (super-1 builds overlap
        # super-0 matmuls)
        st8_pool = ctx.enter_context(tc.tile_pool(name="st8", bufs=1))
        stbf_pool = ctx.enter_context(tc.tile_pool(name="stbf", bufs=1))
        out_pool = ctx.enter_context(tc.tile_pool(name="out", bufs=4))
        psum_pool = ctx.enter_context(tc.tile_pool(name="psum", bufs=1, space="PSUM"))

        # --- PE warmup: un-throttle the HAM clock gate before real work.
        wm = wm_pool.tile([128, 384], BF16, name="wm")
        nc.vector.memset(wm[:], 0.0)
        ps_w = psum_pool.tile([128, 512], F32, name="ps_0_0")
        for _ in range(20):
            nc.tensor.matmul(
                ps_w[:, :256], wm[:, :128], wm[:, 128:384],
                start=True, stop=True, skip_group_check=True,
            )

        # --- input DMAs.  seg pieces on the sync HWDGE ring (group-major so
        # group 0 lands first), g chunks on the scalar ring in consumption
        # order (bf16 chunks then fp8 chunks), base on gpsimd.
        seg_t = seg_pool.tile([128, 4, TOK], F8, name="seg")
        for q in range(N_GRP):
            sl = slice(q * GTOK, (q + 1) * GTOK)
            for jc in range(4):
                nc.sync.dma_start(seg_t[:, jc, sl], seg_d[:, jc, sl])

        gbf_t = gbf_pool.tile([128, NCB, IC, 128], BF16, name="gbf")
        for cb in range(NCB):
            nc.scalar.dma_start(gbf_t[:, cb], gbf_d[:, cb])
        g8_t = g8_pool.tile([128, NC8, 2, IC, 128], F8, name="g8")
        for c8 in range(NC8):
            nc.scalar.dma_start(g8_t[:, c8], g8_d[:, c8])

        base_t = base_pool.tile([128, IC], F32, name="base")
        nc.gpsimd.dma_start(base_t[:], base_d[:])

        # --- step-plane builds (DVE), interleaved by chunk across the two
        # groups of each super so the PE never waits on a late group.
        st8 = [st8_pool.tile([128, NC8, 2, GTOK], F8, name=f"st8_{q}") for q in range(N_GRP)]
        stbf = [stbf_pool.tile([128, NCB, GTOK], BF16, name=f"stbf_{q}") for q in range(N_GRP)]
        for sup in range(2):
            groups = (2 * sup, 2 * sup + 1)
            for kind, c in chunk_order:
                for q in groups:
                    sl = slice(q * GTOK, (q + 1) * GTOK)
                    if kind == "bf":
                        jc, s = 2 + c // 8, c % 8 + 1
                        nc.vector.tensor_scalar(
                            stbf[q][:, c, :], seg_t[:, jc, sl],
                            float(s) - 0.5, None, mybir.AluOpType.is_ge,
                        )
                    else:
                        jb, sp = c // 4, c % 4
                        for b in range(2):
                            s = 2 * sp + 1 + b
                            nc.vector.tensor_scalar(
                                st8[q][:, c, b, :], seg_t[:, jb, sl],
                                float(s) - 0.5, None, mybir.AluOpType.is_ge,
                            )

        # --- matmuls: per super (2 groups), accumulate all 24 chunks into
        # 8 PSUM banks (4 i-chunks x 2 groups); each stationary weight tile
        # feeds 2 consecutive matmuls (the 2 groups). Evacuate on ScalarE
        # (adds base) and DMA out on the sync/scalar rings.
        for sup in range(2):
            groups = (2 * sup, 2 * sup + 1)
            ps = [[psum_pool.tile([128, 512], F32, name=f"ps_{ic}_{qi}")
                   for qi in range(2)] for ic in range(IC)]
            for ci, (kind, c) in enumerate(chunk_order):
                start = ci == 0
                stop = ci == len(chunk_order) - 1
                for ic in range(IC):
                    for qi, q in enumerate(groups):
                        if kind == "bf":
                            nc.tensor.matmul(
                                ps[ic][qi][:],
                                gbf_t[:, c, ic, :],
                                stbf[q][:, c, :],
                                start=start, stop=stop,
                            )
                        else:
                            nc.tensor.matmul(
                                ps[ic][qi][:],
                                g8_t[:, c, :, ic, :],
                                st8[q][:, c, :, :],
                                start=start, stop=stop,
                                perf_mode=mybir.MatmulPerfMode.DoubleRow,
                            )
            for ic in range(IC):
                for qi, q in enumerate(groups):
                    ot = out_pool.tile([128, GTOK], F32, name="ot")
                    nc.scalar.add(ot[:], ps[ic][qi][:], base_t[:, ic : ic + 1])
                    eng = nc.sync if (ic + qi) % 2 == 0 else nc.scalar
                    eng.dma_start(out_d[ic][:, q * GTOK : (q + 1) * GTOK], ot[:])

    nc.compile()
    return nc


def _get_program():
    if "nc" not in _PROGRAM_CACHE:
        _PROGRAM_CACHE["nc"] = _build_program()
    return _PROGRAM_CACHE["nc"]


def kernel(x: np.ndarray, coeffs: np.ndarray) -> np.ndarray:
    assert x.shape == (8, 2048, IN_F) and coeffs.shape == (OUT_F, IN_F, 12)
    t = np.linspace(0.0, 1.0, 10, dtype=np.float32)  # exact knots of reference

    # Segment index per element via the same float32 comparisons the
    # reference uses (bit-exact segment assignment).
    xf = np.ascontiguousarray(x.reshape(-1, IN_F))  # [16384, 512]
    seg = np.zeros(xf.shape, dtype=np.float32)
    for m in range(1, 9):
        seg += (xf >= t[m]).astype(np.float32)
    segT = seg.T  # [512 j, 16384 tok]
    # device layout [p, jc, tok]; values 0..8 are exact in fp8e4m3
    seg_dev = np.ascontiguousarray(
        segT.reshape(4, 128, N_CORES * TOK).transpose(1, 0, 2)
    ).astype(E4NP)

    c = coeffs.astype(np.float32)
    # G[s-1][j, i] = c[i,j,s+2] - c[i,j,s-1]; base[i] = sum_j (c0+c1+c2)
    G = np.empty((8, IN_F, OUT_F), dtype=np.float32)
    for s in range(1, 9):
        G[s - 1] = (c[:, :, s + 2] - c[:, :, s - 1]).T
    base = (c[:, :, 0] + c[:, :, 1] + c[:, :, 2]).sum(axis=1).astype(np.float32)

    JF = 128 * JB8  # j-lanes in fp8
    Gq = np.empty_like(G)
    Gq[:, JF:, :] = G[:, JF:, :].astype(ml_dtypes.bfloat16).astype(np.float32)
    # error-feedback quantization: out uses partial sums of G_s, so carrying
    # the residual keeps the partial-sum error at a single rounding.
    r = np.zeros((JF, OUT_F), dtype=np.float32)
    for s in range(8):
        v = G[s, :JF] + r
        q = v.astype(E4NP).astype(np.float32)
        r = v - q
        Gq[s, :JF] = q
    # mean-centering: fold the per-lane mean partial-sum error into base
    # (seg is uniform over its 9 values for uniform x).
    E = np.cumsum(G, axis=0) - np.cumsum(Gq, axis=0)  # [8, j, i]
    base_adj = base + (E.sum(axis=0) / 9.0).sum(axis=0)

    # fp8 weights: g8[p, c8 = jb*4+sp, b, ic, m] = Gq[2*sp+b, jb*128+p, ic*128+m]
    Gf = Gq[:, :JF, :].reshape(4, 2, JB8, 128, IC, 128)  # [sp, b, jb, p, ic, m]
    g8 = np.ascontiguousarray(Gf.transpose(3, 2, 0, 1, 4, 5).reshape(128, NC8, 2, IC, 128)).astype(E4NP)
    # bf16 weights: gbf[p, cb = jcb*8 + (s-1), ic, m] = Gq[s-1, (2+jcb)*128+p, ...]
    Gb = Gq[:, JF:, :].reshape(8, 4 - JB8, 128, IC, 128)  # [s, jcb, p, ic, m]
    gbf = np.ascontiguousarray(Gb.transpose(2, 1, 0, 3, 4).reshape(128, NCB, IC, 128)).astype(ml_dtypes.bfloat16)
    base_tile = np.ascontiguousarray(base_adj.reshape(IC, 128).T)  # [p, ic]

    in_maps = []
    for core in range(N_CORES):
        sl = slice(core * TOK, (core + 1) * TOK)
        in_maps.append(
            {
                "seg": np.ascontiguousarray(seg_dev[:, :, sl]),
                "g8": g8,
                "gbf": gbf,
                "base": base_tile,
            }
        )

    nc = _get_program()
    res = run_bass_kernel_spmd(nc, in_maps, core_ids=list(range(N_CORES)))
    # out[ic, p, tok] -> [tok, i]
    out = np.stack(
        [
            np.ascontiguousarray(res.results[core]["out"].reshape(OUT_F, TOK).T)
            for core in range(N_CORES)
        ]
    )
    return out.astype(np.float32)
